# revision 8
# baseline (speedup 1.0000x reference)
"""Trainium2 Bass kernel for the differentiable LogicLayer forward pass.

Math (per output neuron j with a = x[:, idx_a[j]], b = x[:, idx_b[j]]):
    w      = softmax(weights[j])          # [14]  (computed on HOST)
    coeffs = w @ OP_COEFFS                # [4] -> c0, ca, cb, cab
    out[:, j] = c0 + ca*a + cb*b + cab*a*b

Sharding: data-parallel over batch across 8 NeuronCores (1024 rows each);
weights / indices replicated.  Per core the kernel works feature-major:
partition p holds output neuron pi[t*128 + p] where pi sorts neurons by
(ineligible, idx_a); the free dim holds the 1024-sample batch shard.

The a-side gather runs on the Tensor engine: the batch shard lives in
SBUF as 32 blocks of 128 feature rows and each group of 128
pi-consecutive neurons pulls its a-rows with one-hot stationary (fp8)
matmuls (sorted neurons touch 1-2 blocks).  The psum is evacuated once
by ACT with the fused affine u = cab*a + cb.

kappa-chunks (|ca/cab| <= KMAX, ~83% of neurons): the remaining terms
use out = u*(b + kappa) + gamma with kappa = ca/cab and
gamma = c0 - kappa*cb -- two cheap 4x-mode DVE tensor_scalar passes plus
one fp16 tensor_tensor, no second psum read.  Classic chunks (the ones
holding ineligible neurons) use the always-correct two-evac pipeline
v = ca*a + c0 (ACT or DVE, the load-balance knob) and out = u*b + v.

The b-side gather stays on the SWDGE dma_gather with pi-permuted
indices.  Output is stored fp16 in pi order and unpermuted/widened on
the host.
"""

import sys

import numpy as np

try:  # the axon sitecustomize usually provides concourse already
    import concourse  # noqa: F401
except ImportError:  # pragma: no cover
    sys.path.insert(0, "/opt/trn_rl_repo")

import concourse.bacc as bacc
import concourse.mybir as mybir
import concourse.tile as tile
from concourse.bass import MemorySpace
from concourse.bass_utils import run_bass_kernel_spmd
from concourse.library_config import mlp as mlp_library

F32 = mybir.dt.float32
F16 = mybir.dt.float16
F8 = mybir.dt.float8e4
I16 = mybir.dt.int16

NCORES = 8
BATCH, IN_DIM, OUT_DIM, NOPS = 8192, 4096, 16384, 14
B = BATCH // NCORES            # 1024 batch rows per core
NJC = 512                      # output neurons per gather chunk
NCH = OUT_DIM // NJC           # 32 chunks
SL = NJC // 128                # 4 partition-slices (groups) per chunk
NT = OUT_DIM // 128            # 128 neuron groups
NBLK = IN_DIM // 128           # 32 xT row-blocks resident in SBUF
KMAX = 3.0                     # |ca/cab| bound for the kappa pipeline
OH_FP8 = False                  # one-hot stationaries in fp8 (halved DMA)
# among classic chunks' groups, how many v-evacs run on ACT (rest DVE)
V_ON_ACT = 24

_OP_COEFFS = np.array([
    [0,  0,  0,  1],
    [0,  1,  0, -1],
    [0,  1,  0,  0],
    [0,  0,  1, -1],
    [0,  0,  1,  0],
    [0,  1,  1, -2],
    [0,  1,  1, -1],
    [1, -1, -1,  1],
    [1, -1, -1,  2],
    [1,  0, -1,  0],
    [1,  0, -1,  1],
    [1, -1,  0,  0],
    [1, -1,  0,  1],
    [1,  0,  0, -1],
], dtype=np.float32)


def build_program(plan, n_kappa_chunks):
    """Build + compile the per-core Bass program.

    plan[g] = tuple of xts 128-row blocks the g-th sorted neuron group
    draws its a-rows from.  The first n_kappa_chunks chunks use the
    kappa pipeline; the rest use the classic two-evac pipeline.
    """
    nmm = sum(len(bs) for bs in plan)
    oh_dt = F8 if OH_FP8 else F16

    nc = bacc.Bacc("TRN2", target_bir_lowering=False, debug=False,
                   num_devices=NCORES)

    xt = nc.dram_tensor("xt", [IN_DIM, B], F16, kind="ExternalInput")
    xtp = nc.dram_tensor("xtp", [128, NBLK, B], F16, kind="ExternalInput")
    oh = nc.dram_tensor("oh", [128, nmm, 128], oh_dt, kind="ExternalInput")
    # per-neuron scalars (pi order): [p, t, 0..3] = cab, cb, k2, k3
    # where (k2, k3) = (kappa, gamma) for kappa chunks, (ca, c0) otherwise
    coef = nc.dram_tensor("coef", [128, NT, 4], F32, kind="ExternalInput")
    idxb = nc.dram_tensor("idxb", [128, OUT_DIM // 16], I16, kind="ExternalInput")
    # p-major output: [p, t, b] holds neuron pi[t*128+p]
    out = nc.dram_tensor("out", [128, NT, B], F16, kind="ExternalOutput")

    out_r = out.ap()

    mult = mybir.AluOpType.mult
    add = mybir.AluOpType.add
    ident = mybir.ActivationFunctionType.Identity

    with tile.TileContext(nc) as tc:
        nc.gpsimd.load_library(mlp_library)
        with (
            tc.tile_pool(name="const", bufs=1) as cpool,
        ):
            # small inputs first: the first b-gather must not queue behind
            # the big resident loads
            ib_sb = cpool.tile([128, OUT_DIM // 16], I16)
            nc.sync.dma_start(ib_sb[:], idxb.ap())
            cf = cpool.tile([128, NT, 4], F32)
            nc.sync.dma_start(cf[:], coef.ap())

            # resident tiles; pieces stream in during the first chunks
            xts = cpool.tile([128, NBLK, B], F16)
            oh_sb = cpool.tile([128, nmm, 128], oh_dt)
            qb = NBLK // 8
            qm = (nmm + 7) // 8

            # interleave xts blocks and oh stationaries so both stay
            # ahead of the consuming matmuls (same Q, FIFO order)
            for q in range(8):
                nc.sync.dma_start(xts[:, q * qb:(q + 1) * qb],
                                  xtp.ap()[:, q * qb:(q + 1) * qb])
                lo, hi = q * qm, min((q + 1) * qm, nmm)
                if lo < hi:
                    nc.sync.dma_start(oh_sb[:, lo:hi], oh.ap()[:, lo:hi])

            cab = cf[:, :, 0]
            cb_ = cf[:, :, 1]
            k2 = cf[:, :, 2]
            k3 = cf[:, :, 3]

            # ---- main loop: PE a-gather, SWDGE b-gather, combine, store ----
            with (
                tc.tile_pool(name="gb", bufs=3) as bpool,
                tc.tile_pool(name="go", bufs=2) as opool,
                tc.tile_pool(name="uv", bufs=2) as uvpool,
                tc.tile_pool(name="ps", bufs=4,
                             space=MemorySpace.PSUM) as ppool,
            ):
                w16 = NJC // 16  # idx columns per chunk
                mi = 0           # running matmul index into oh
                vact = 0         # classic-group v-on-ACT rotation counter
                n_classic_groups = (NCH - n_kappa_chunks) * SL
                for ci in range(NCH):
                    is_kappa = ci < n_kappa_chunks
                    bt = bpool.tile([128, SL, B], F16)
                    nc.gpsimd.dma_gather(
                        bt[:], xt.ap(), ib_sb[:, ci * w16:(ci + 1) * w16],
                        NJC, NJC, B)
                    ot = opool.tile([128, SL, B], F16)
                    u = uvpool.tile([128, SL, B], F16, tag="u")
                    v = uvpool.tile([128, SL, B], F16, tag="v")
                    for s in range(SL):
                        g = ci * SL + s
                        blocks = plan[g]
                        pt = ppool.tile([128, B], F32, tag="ps")
                        for k, c in enumerate(blocks):
                            # PSUM bank limit: 512 fp32 per matmul
                            for q in range(2):
                                qo = q * (B // 2)
                                nc.tensor.matmul(
                                    pt[:, qo:qo + B // 2], oh_sb[:, mi],
                                    xts[:, c, qo:qo + B // 2],
                                    start=(k == 0),
                                    stop=(k == len(blocks) - 1))
                            mi += 1
                        # u = cab*a + cb  (ACT evac of psum, fused affine)
                        nc.scalar.activation(u[:, s], pt[:], ident,
                                             bias=cb_[:, g:g + 1],
                                             scale=cab[:, g:g + 1])
                        if is_kappa:
                            # t = b + kappa  (DVE 2x fp16)
                            nc.vector.tensor_scalar(
                                v[:, s], bt[:, s], k2[:, g:g + 1], None,
                                op0=add)
                        else:
                            # v = ca*a + c0  (second evac, ACT or DVE)
                            vact += V_ON_ACT
                            if vact >= n_classic_groups:
                                vact -= n_classic_groups
                                nc.scalar.activation(v[:, s], pt[:], ident,
                                                     bias=k3[:, g:g + 1],
                                                     scale=k2[:, g:g + 1])
                            else:
                                nc.vector.tensor_scalar(
                                    v[:, s], pt[:], k2[:, g:g + 1],
                                    k3[:, g:g + 1], op0=mult, op1=add)
                    for h in range(SL // 2):
                        sl2 = slice(h * 2, h * 2 + 2)
                        wt = uvpool.tile([128, 2, B], F16, tag="w")
                        if is_kappa:
                            # w = u*(b+kappa), then out = w + gamma
                            nc.vector.tensor_tensor(wt[:], u[:, sl2],
                                                    v[:, sl2], op=mult)
                            for s2 in range(2):
                                s = h * 2 + s2
                                g = ci * SL + s
                                nc.vector.tensor_scalar(
                                    ot[:, s], wt[:, s2], k3[:, g:g + 1],
                                    None, op0=add)
                        else:
                            # out = u*b + v  (DVE, fp16 2x)
                            nc.vector.tensor_tensor(wt[:], u[:, sl2],
                                                    bt[:, sl2], op=mult)
                            nc.vector.tensor_tensor(ot[:, sl2], wt[:],
                                                    v[:, sl2], op=add)
                    nc.sync.dma_start(out_r[:, ci * SL:(ci + 1) * SL], ot[:])
                assert mi == nmm

    nc.compile()
    return nc


_PROGRAMS = {}
_NEEDS_INPUTS = True


def _coeffs(weights):
    w = np.asarray(weights, dtype=np.float64)
    e = np.exp(w - w.max(axis=-1, keepdims=True))
    sm = e / e.sum(axis=-1, keepdims=True)
    return sm @ _OP_COEFFS.astype(np.float64)      # [OUT_DIM, 4]


def _eligible(co):
    c0, ca, cb, cab = co[:, 0], co[:, 1], co[:, 2], co[:, 3]
    with np.errstate(divide="ignore", invalid="ignore"):
        kap = ca / cab
    kap = np.nan_to_num(kap, nan=np.inf, posinf=np.inf, neginf=-np.inf)
    gam = c0 - kap * cb
    return (np.abs(kap) <= KMAX) & (np.abs(gam) <= 8.0), kap, gam


def _make_plan(idx_a, weights):
    """Two-segment permutation (kappa-eligible first) + per-group block
    lists + number of kappa chunks."""
    ia = np.asarray(idx_a).astype(np.int64)
    co = _coeffs(weights)
    elig, _, _ = _eligible(co)
    n_kappa_chunks = int(elig.sum()) // NJC
    # first n_kappa_chunks*NJC slots must be eligible neurons: put the
    # ineligible ones last (each segment sorted by idx_a)
    pi = np.argsort(ia + ((~elig).astype(np.int64) << 32), kind="stable")
    ia_s = ia[pi]
    plan = []
    for g in range(NT):
        rows = ia_s[g * 128:(g + 1) * 128]
        plan.append(tuple(sorted(set(int(r) // 128 for r in rows))))
    return pi, ia_s, tuple(plan), n_kappa_chunks


def _get_program(x=None, weights=None, idx_a=None, idx_b=None):
    _, _, plan, nkc = _make_plan(idx_a, weights)
    if (plan, nkc) not in _PROGRAMS:
        _PROGRAMS[(plan, nkc)] = build_program(plan, nkc)
    return _PROGRAMS[(plan, nkc)]


def _wrap_idx(idx):
    """[OUT_DIM] int -> SWDGE-wrapped int16 [128, OUT_DIM//16].

    Per NJC-chunk c, columns [c*NJC//16:(c+1)*NJC//16] hold that chunk's
    indices with index i at (partition i%16, column i//16), replicated
    across the 8 groups of 16 partitions (one per Q7 core).
    """
    i16 = idx.astype(np.int16).reshape(NCH, NJC // 16, 16)
    w = i16.transpose(2, 0, 1).reshape(16, NCH * (NJC // 16))
    return np.ascontiguousarray(np.tile(w, (8, 1)))


def _build_oh(ia_s, plan):
    """One-hot stationaries [128, nmm, 128]: column m of matmul (g, c)
    selects xts row ia_s[g*128+m] when it lies in block c, else zero."""
    nmm = sum(len(bs) for bs in plan)
    if OH_FP8:
        import ml_dtypes
        dt = ml_dtypes.float8_e4m3
    else:
        dt = np.float16
    ohm = np.zeros((128, nmm, 128), dtype=dt)
    mi = 0
    cols = np.arange(128)
    for g in range(NT):
        rows = ia_s[g * 128:(g + 1) * 128]
        for c in plan[g]:
            rel = rows - 128 * c
            m = (rel >= 0) & (rel < 128)
            ohm[rel[m], mi, cols[m]] = 1.0
            mi += 1
    return ohm


def prepare_in_maps(x, weights, idx_a, idx_b):
    x = np.asarray(x, dtype=np.float32)
    idx_a = np.asarray(idx_a)
    idx_b = np.asarray(idx_b)

    pi, ia_s, plan, nkc = _make_plan(idx_a, weights)
    co = _coeffs(weights)[pi]                      # [OUT_DIM, 4] pi order
    elig, kap, gam = _eligible(co)
    c0, ca, cb, cab = co[:, 0], co[:, 1], co[:, 2], co[:, 3]

    in_kappa = np.arange(OUT_DIM) < nkc * NJC
    assert elig[in_kappa].all()
    k2 = np.where(in_kappa, np.where(elig, kap, 0.0), ca)
    k3 = np.where(in_kappa, np.where(elig, gam, 0.0), c0)

    ohm = _build_oh(ia_s, plan)
    cf = np.stack([cab, cb, k2, k3], axis=-1).astype(np.float32)
    cf = np.ascontiguousarray(
        cf.reshape(NT, 128, 4).transpose(1, 0, 2))  # [128, NT, 4]
    ib = _wrap_idx(idx_b[pi])

    global _PI
    _PI = pi
    x16 = x.astype(np.float16)
    in_maps = []
    for c in range(NCORES):
        xt = np.ascontiguousarray(x16[c * B:(c + 1) * B].T)   # [4096, B]
        xtp = np.ascontiguousarray(
            xt.reshape(NBLK, 128, B).transpose(1, 0, 2))
        in_maps.append({"xt": xt, "xtp": xtp, "oh": ohm, "coef": cf,
                        "idxb": ib})
    return in_maps


_PI = None


def assemble_output(results):
    out = np.empty((BATCH, OUT_DIM), dtype=np.float32)
    for c in range(NCORES):
        dev = results[c]["out"]  # [128, NT, B], neuron pi[t*128+p]
        out[c * B:(c + 1) * B, _PI] = \
            dev.transpose(2, 1, 0).reshape(B, OUT_DIM)
    return out


def kernel(x, weights, idx_a, idx_b):
    nc = _get_program(weights=weights, idx_a=idx_a)
    in_maps = prepare_in_maps(x, weights, idx_a, idx_b)
    res = run_bass_kernel_spmd(nc, in_maps, list(range(NCORES)))
    return assemble_output(res.results)
